# revision 1
# baseline (speedup 1.0000x reference)
"""CGC (Customized Gate Control) MoE layer on 8 Trainium2 NeuronCores.

Strategy: data-parallel over batch. B=4096 is split into 8 shards of 512
rows; every core holds all 8 expert MLPs (weights replicated in its
in_map) and computes the full layer for its shard — no collectives.

Per-core dataflow (BL=512 local batch):
  - x inputs are PE-transposed into xT [D-part, B-free] layout.
  - Expert layer 1: hT[H1,B] = relu(W1.T-free matmul) with per-partition
    bias fused into the ScalarE activation.
  - Expert layer 2: out[B,H2] natural layout; b2 is broadcast to a
    [128,H2] tile once per expert (rank-1 matmul ones.T @ b2), then the
    bias-add + relu run on VectorE (add + max). The final expert instead
    appends the rank-1 matmul to its PSUM group and relus on ScalarE,
    which shortens the kernel-tail dependency chain.
  - Gates: logits via matmul (lhsT=Wg, moving xT), bias on ScalarE,
    PE-transpose back to [B-part, K], softmax along the free dim.
  - Gated combine: single-instruction MAC on VectorE
    (scalar_tensor_tensor: acc = oe * gw[:,col] + acc).
  - x for the next domain is prefetched during the current domain's
    first expert; W2 loads are split into 512KB slabs and deferred past
    the W1 slabs they'd otherwise delay.
Matmuls run as float32r (full-rate fp32 at moving-dim >= 256, ~fp32
accuracy: 2.3e-4 max rel err vs the fp32 reference on hardware).
"""

import numpy as np

import concourse.tile as tile
from concourse import bacc, mybir
from concourse.bass_utils import run_bass_kernel_spmd

N_CORES = 8
B = 4096
BL = B // N_CORES  # 512 rows per core
D = 1024
H1 = 1024
H2 = 512
DOM = 3
NES = 2
NSH = 2
E_SPEC = DOM * NES  # 6
GATE_K = NES + NSH  # 4
TOTAL_E = E_SPEC + NSH  # 8

F32 = mybir.dt.float32
F32R = mybir.dt.float32r
AX = mybir.AxisListType
AF = mybir.ActivationFunctionType
ALU = mybir.AluOpType

NBT = BL // 128  # 4 batch tiles per core
NKD = D // 128   # 8 contraction tiles over D
NKH = H1 // 128  # 8 contraction tiles over H1
NMH = H1 // 128  # 8 output tiles over H1


def _build_nc(mm_dt=F32R):
    from contextlib import ExitStack

    nc = bacc.Bacc("TRN2", target_bir_lowering=False, debug=False)

    xs = [
        nc.dram_tensor(n, [BL, D], F32, kind="ExternalInput")
        for n in ("x0", "x1", "x2", "x_shared")
    ]
    W1s = nc.dram_tensor("W1s", [E_SPEC, D, H1], mm_dt, kind="ExternalInput")
    b1s = nc.dram_tensor("b1s", [E_SPEC, H1], F32, kind="ExternalInput")
    W2s = nc.dram_tensor("W2s", [E_SPEC, H1, H2], mm_dt, kind="ExternalInput")
    b2s = nc.dram_tensor("b2s", [E_SPEC, H2], mm_dt, kind="ExternalInput")
    W1h = nc.dram_tensor("W1h", [NSH, D, H1], mm_dt, kind="ExternalInput")
    b1h = nc.dram_tensor("b1h", [NSH, H1], F32, kind="ExternalInput")
    W2h = nc.dram_tensor("W2h", [NSH, H1, H2], mm_dt, kind="ExternalInput")
    b2h = nc.dram_tensor("b2h", [NSH, H2], mm_dt, kind="ExternalInput")
    Wg = nc.dram_tensor("Wg", [DOM, D, GATE_K], mm_dt, kind="ExternalInput")
    bg = nc.dram_tensor("bg", [DOM, GATE_K], F32, kind="ExternalInput")
    Wsg = nc.dram_tensor("Wsg", [D, TOTAL_E], mm_dt, kind="ExternalInput")
    bsg = nc.dram_tensor("bsg", [TOTAL_E], F32, kind="ExternalInput")
    ys = [
        nc.dram_tensor(n, [BL, H2], F32, kind="ExternalOutput")
        for n in ("y0", "y1", "y2", "ysh")
    ]


    with tile.TileContext(nc) as tc, ExitStack() as ctx:
        p_const = ctx.enter_context(tc.tile_pool(name="const", bufs=1))
        p_xstage = ctx.enter_context(tc.tile_pool(name="xstage", bufs=2))
        p_xT = ctx.enter_context(tc.tile_pool(name="xT", bufs=2))
        p_w1 = ctx.enter_context(tc.tile_pool(name="w1", bufs=4))
        p_w2 = ctx.enter_context(tc.tile_pool(name="w2", bufs=1))
        p_h = ctx.enter_context(tc.tile_pool(name="hT", bufs=2))
        p_oe = ctx.enter_context(tc.tile_pool(name="oe", bufs=2))
        p_osh = ctx.enter_context(tc.tile_pool(name="osh", bufs=1))
        p_acc = ctx.enter_context(tc.tile_pool(name="acc", bufs=1))
        p_bias = ctx.enter_context(tc.tile_pool(name="bias", bufs=2))
        p_gw = ctx.enter_context(tc.tile_pool(name="gw", bufs=1))
        p_gt = ctx.enter_context(tc.tile_pool(name="gt", bufs=2))
        p_sm = ctx.enter_context(tc.tile_pool(name="sm", bufs=3))
        p_tmp = ctx.enter_context(tc.tile_pool(name="tmp", bufs=2))
        ps_h = ctx.enter_context(tc.tile_pool(name="psh", bufs=2, space="PSUM"))
        ps_o = ctx.enter_context(tc.tile_pool(name="pso", bufs=3, space="PSUM"))
        ps_t = ctx.enter_context(tc.tile_pool(name="pst", bufs=3, space="PSUM"))

        # Build identity/ones on-chip: no DMA ahead of the x transfers.
        ident_sb = p_const.tile([128, 128], F32)
        nc.gpsimd.memset(ident_sb, 0.0)
        nc.gpsimd.affine_select(
            out=ident_sb,
            in_=ident_sb,
            compare_op=ALU.not_equal,
            fill=1.0,
            base=0,
            pattern=[[-1, 128]],
            channel_multiplier=1,
        )
        identr_sb = p_const.tile([128, 128], mm_dt)
        nc.scalar.copy(out=identr_sb, in_=ident_sb)
        onesf_sb = p_const.tile([1, 128], F32)
        nc.gpsimd.memset(onesf_sb, 1.0)
        ones_sb = p_const.tile([1, 128], mm_dt)
        nc.scalar.copy(out=ones_sb, in_=onesf_sb)
        # PE warm-up: harmless matmuls on the identity while the first x/W
        # DMAs are in flight, so the HAM clock gate opens before real work.
        for _ in range(16):
            pw = ps_t.tile([128, 128], F32, tag="pt", name="pw")
            nc.tensor.matmul(pw, lhsT=identr_sb, rhs=identr_sb, start=True, stop=True)
        def transpose_x(x_dram):
            """[BL, D] natural -> xT tile [128, NKD, BL] (d on partitions).

            j-outer so slab j is complete (and consumable by L1/gates)
            after only NBT transposes; 4 transposes share one PSUM bank and
            drain with a single contiguous ACT copy.
            """
            xT = p_xT.tile([128, NKD, BL], mm_dt, tag="xT")
            xsts = x_dram
            for j in range(NKD):
                pt = ps_t.tile([128, BL], mm_dt, tag="pt")
                for bt in range(NBT):
                    nc.tensor.transpose(
                        pt[:, bt * 128 : (bt + 1) * 128],
                        xsts[bt][:, j * 128 : (j + 1) * 128],
                        identr_sb,
                    )
                nc.scalar.copy(out=xT[:, j, :], in_=pt)
            return xT

        def load_xstage(x_dram, bts=range(NBT)):
            xsts = []
            for bt in bts:
                xst = p_xstage.tile(
                    [128, D], mm_dt, tag=f"xst{bt}", name=f"xst{bt}", bufs=1
                )
                nc.sync.dma_start(
                    out=xst,
                    in_=x_dram[bt * 128 : (bt + 1) * 128, :].bitcast(mm_dt),
                )
                xsts.append(xst)
            return xsts

        def compute_gate(xT, wg_2d, bias_1d, K, tag):
            """softmax(x @ Wg + bg) -> gw tile [128, NBT, K] (b on partitions)."""
            wg_sb = p_sm.tile([128, NKD, K], mm_dt, tag=f"wg{K}")
            nc.sync.dma_start(
                out=wg_sb, in_=wg_2d.rearrange("(kt p) k -> p kt k", p=128)
            )
            bg_sb = p_sm.tile([K, 1], F32, tag=f"bg{K}")
            nc.sync.dma_start(
                out=bg_sb, in_=bias_1d.rearrange("(k one) -> k one", one=1)
            )
            pg = ps_t.tile([K, BL], F32, tag="pt")
            for kt in range(NKD):
                nc.tensor.matmul(
                    pg,
                    lhsT=wg_sb[:, kt, :],
                    rhs=xT[:, kt, :],
                    start=(kt == 0),
                    stop=(kt == NKD - 1),
                )
            glT = p_gt.tile([K, BL], F32, tag="glT")
            nc.scalar.activation(
                out=glT, in_=pg, func=AF.Identity, bias=bg_sb, scale=1.0
            )
            gw = p_gw.tile([128, NBT, K], F32, tag=tag)
            for bt in range(NBT):
                ptg = ps_t.tile([128, K], F32, tag="pt")
                nc.tensor.transpose(
                    ptg, glT[:, bt * 128 : (bt + 1) * 128], ident_sb[:K, :K]
                )
                nm = p_sm.tile([128, 1], F32, tag="nm")
                nc.vector.reduce_max(out=nm, in_=ptg, axis=AX.X, negate=True)
                esb = p_sm.tile([128, K], F32, tag="esb")
                nc.scalar.activation(
                    out=esb, in_=ptg, func=AF.Exp, bias=nm, scale=1.0
                )
                ssb = p_sm.tile([128, 1], F32, tag="ssb")
                nc.vector.reduce_sum(out=ssb, in_=esb, axis=AX.X)
                rsb = p_sm.tile([128, 1], F32, tag="rsb")
                nc.vector.reciprocal(out=rsb, in_=ssb)
                nc.vector.tensor_scalar_mul(gw[:, bt, :], esb, rsb)
            return gw

        def expert(xT, w1_2d, b1_1d, w2_2d, b2_1d, out_pool, tag, bias_mm=False):
            """Two-layer MLP: relu(relu(x@W1+b1)@W2+b2) -> [128, NBT, H2]."""
            b1_sb = p_bias.tile([128, NMH], F32, tag="b1")
            nc.sync.dma_start(
                out=b1_sb, in_=b1_1d.rearrange("(mt p) -> p mt", p=128)
            )
            b2_sb = p_bias.tile([1, H2], mm_dt, tag="b2")
            nc.sync.dma_start(
                out=b2_sb, in_=b2_1d.rearrange("(one o) -> one o", one=1)
            )
            if not bias_mm:
                bb = ps_t.tile([128, H2], F32, tag="pt", name="bb")
                nc.tensor.matmul(
                    bb, lhsT=ones_sb, rhs=b2_sb, start=True, stop=True
                )
                b2bc = p_tmp.tile([128, H2], F32, tag="b2bc", name="b2bc")
                nc.scalar.copy(out=b2bc, in_=bb)
            w1r = w1_2d.rearrange("(kt p) h -> p kt h", p=128)
            hT = p_h.tile([128, NMH, BL], mm_dt, tag="hT")
            for mt in range(NMH):
                w1_sb = p_w1.tile([128, NKD, 128], mm_dt, tag="w1")
                nc.sync.dma_start(
                    out=w1_sb, in_=w1r[:, :, mt * 128 : (mt + 1) * 128]
                )

                ph = ps_h.tile([128, BL], F32, tag="ph")
                for kt in range(NKD):
                    nc.tensor.matmul(
                        ph,
                        lhsT=w1_sb[:, kt, :],
                        rhs=xT[:, kt, :],
                        start=(kt == 0),
                        stop=(kt == NKD - 1),
                    )
                nc.scalar.activation(
                    out=hT[:, mt, :],
                    in_=ph,
                    func=AF.Relu,
                    bias=b1_sb[:, mt : mt + 1],
                    scale=1.0,
                )
            w2_sb = p_w2.tile([128, NKH, H2], mm_dt, tag="w2")
            w2r = w2_2d.rearrange("(kt p) o -> p kt o", p=128)
            for g in range(0, NKH, 2):
                nc.sync.dma_start(
                    out=w2_sb[:, g : g + 2, :], in_=w2r[:, g : g + 2, :]
                )
            oe = out_pool.tile([128, NBT, H2], F32, tag=tag)
            for bt in range(NBT):
                po = ps_o.tile([128, H2], F32, tag="po")
                po2 = po
                for kt in range(NKH):
                    nc.tensor.matmul(
                        po,
                        lhsT=hT[:, kt, bt * 128 : (bt + 1) * 128],
                        rhs=w2_sb[:, kt, :],
                        start=(kt == 0),
                        stop=(False if bias_mm else kt == NKH - 1),
                    )
                if bias_mm:
                    nc.tensor.matmul(
                        po2, lhsT=ones_sb, rhs=b2_sb, start=False, stop=True
                    )
                    nc.scalar.activation(out=oe[:, bt, :], in_=po2, func=AF.Relu)
                else:
                    nc.vector.tensor_tensor(oe[:, bt, :], po, b2bc, ALU.add)
                    nc.vector.tensor_scalar_max(oe[:, bt, :], oe[:, bt, :], 0.0)
            return oe

        accs = [None] * 4

        def accumulate(acc_idx, oe, gw, col, first):
            acc = accs[acc_idx]
            for bt in range(NBT):
                if first:
                    nc.vector.tensor_scalar_mul(
                        acc[:, bt, :], oe[:, bt, :], gw[:, bt, col : col + 1]
                    )
                else:
                    nc.vector.scalar_tensor_tensor(
                        out=acc[:, bt, :],
                        in0=oe[:, bt, :],
                        scalar=gw[:, bt, col : col + 1],
                        in1=acc[:, bt, :],
                        op0=ALU.mult,
                        op1=ALU.add,
                    )

        # ---- shared phase: shared experts kept resident, shared gate ----
        xT_sh = transpose_x(load_xstage(xs[3]))
        gws = compute_gate(xT_sh, Wsg[:], bsg[:], TOTAL_E, tag="gws")
        osh = []
        xsts_next = None
        for j in range(NSH):
            o = expert(
                xT_sh, W1h[j], b1h[j], W2h[j], b2h[j], p_osh, tag=f"osh{j}"
            )
            osh.append(o)
            if j == 0:
                xsts_next = load_xstage(xs[0])
        accs[3] = p_acc.tile([128, NBT, H2], F32, tag="acc3", name="acc3")
        accumulate(3, osh[0], gws, E_SPEC + 0, first=True)
        accumulate(3, osh[1], gws, E_SPEC + 1, first=False)

        # ---- domain phases ----
        for d in range(DOM):
            xT_d = transpose_x(xsts_next)
            gw_d = compute_gate(xT_d, Wg[d], bg[d], GATE_K, tag=f"gw{d}")
            accs[d] = p_acc.tile(
                [128, NBT, H2], F32, tag=f"acc{d}", name=f"acc{d}"
            )
            accumulate(d, osh[0], gw_d, NES + 0, first=True)
            accumulate(d, osh[1], gw_d, NES + 1, first=False)
            for i in range(NES):
                e = d * NES + i
                oe = expert(
                    xT_d, W1s[e], b1s[e], W2s[e], b2s[e], p_oe, tag="oe",
                    bias_mm=(e == E_SPEC - 1),
                )
                if i == 0 and d < DOM - 1:
                    xsts_next = load_xstage(xs[d + 1])
                accumulate(d, oe, gw_d, i, first=False)
                accumulate(3, oe, gws, e, first=False)
            yr = ys[d][:].rearrange("(bt p) o -> bt p o", p=128)
            for bt in range(NBT):
                nc.sync.dma_start(out=yr[bt], in_=accs[d][:, bt, :])
        yr3 = ys[3][:].rearrange("(bt p) o -> bt p o", p=128)
        for bt in range(NBT):
            nc.sync.dma_start(out=yr3[bt], in_=accs[3][:, bt, :])

    nc.compile()
    return nc


_NC_CACHE = {}


def _get_nc(mm_dt=F32R):
    key = str(mm_dt)
    if key not in _NC_CACHE:
        _NC_CACHE[key] = _build_nc(mm_dt)
    return _NC_CACHE[key]


def kernel(**inputs):
    return run_kernel(inputs)


def run_kernel(inputs, mm_dt=F32R, trace=False):
    nc = _get_nc(mm_dt)
    shard_names = ("x0", "x1", "x2", "x_shared")
    full = {k: np.ascontiguousarray(np.asarray(v, dtype=np.float32)) for k, v in inputs.items()}
    in_maps = []
    for c in range(N_CORES):
        m = {}
        for k, v in full.items():
            if k in shard_names:
                m[k] = v[c * BL : (c + 1) * BL]
            else:
                m[k] = v
        in_maps.append(m)
    res = run_bass_kernel_spmd(nc, in_maps, list(range(N_CORES)), trace=trace)
    outs = []
    for name in ("y0", "y1", "y2", "ysh"):
        outs.append(
            np.concatenate([res.results[c][name] for c in range(N_CORES)], axis=0)
        )
    out = tuple(outs)
    if trace:
        return out, res
    return out



# revision 3
# speedup vs baseline: 1.3883x; 1.3883x over previous
"""CGC (Customized Gate Control) MoE layer on 8 Trainium2 NeuronCores.

Strategy: data-parallel over batch. B=4096 is split into 8 shards of 512
rows; every core holds all 8 expert MLPs (weights replicated in its
in_map) and computes the full layer for its shard - no collectives.

All expert/gate matmuls run as fp8e4 DoubleRow with 3-term error
compensation. Each f32 operand A is host-split into A_hi = fp8(s*A) and
A_lo = fp8(s*A - A_hi); the product x@W is computed as

    x_hi@(W_hi + W_lo) + x_lo@W_hi        (lo*lo term dropped)

by packing hi/lo pairs into the two DoubleRow contraction slots:
  instr1: lhsT slots (W_hi_k, W_lo_k)   rhs slots (x_hi_k, x_hi_k) [stride-0]
  instr2: lhsT slots (W_hi_k, W_hi_k+1) rhs slots (x_lo_k, x_lo_k+1)
Full K=1024 costs 12 DoubleRow instructions per [128,256] output chunk
(DoubleRow = 0.5 cycles/output-row): a 1.33x Tensor speedup over exact
fp32r at ~0.3% relative error.

Per-core dataflow (BL=512 local batch):
  - x arrives pre-transposed/quantized from host as [128, 8kt, 2hl, BL]
    fp8 tiles (no PE transposes at all).
  - L1: chunks drain via ACT (relu + per-partition bias + scale) to a
    f32 temp; GpSimd converts to h_hi fp8, DVE computes h_lo fp8; both
    land in the interleaved hT tile consumed by L2.
  - L2: out natural [b, H2]; b2 enters the PSUM group as a rank-1 bf16
    matmul (ones.T @ b2); drains on ACT as relu -> bf16.
  - Gates: logits computed directly in [b-part, K-free] orientation
    (stationary = xT slots, moving = Wg slots, rank-1 bf16 bias append),
    so no transposes; softmax runs off PSUM on DVE/ACT.
  - Gated combine: single-instruction MAC on VectorE in bf16
    (scalar_tensor_tensor, 2x throughput mode for 2-byte dtypes).
  - Expert schedule is software-pipelined (L2 of expert e runs after L1
    of expert e+1) so PE never waits on the L1 drain chain.
Outputs are bf16, upcast to f32 on host.
"""

import numpy as np
import ml_dtypes

import concourse.tile as tile
from concourse import bacc, mybir
from concourse.bass_utils import run_bass_kernel_spmd

N_CORES = 8
B = 4096
BL = B // N_CORES  # 512 rows per core
D = 1024
H1 = 1024
H2 = 512
DOM = 3
NES = 2
NSH = 2
E_SPEC = DOM * NES  # 6
GATE_K = NES + NSH  # 4
TOTAL_E = E_SPEC + NSH  # 8

F32 = mybir.dt.float32
F32R = mybir.dt.float32r
F8 = mybir.dt.float8e4
BF16 = mybir.dt.bfloat16
AX = mybir.AxisListType
AF = mybir.ActivationFunctionType
ALU = mybir.AluOpType
DR = mybir.MatmulPerfMode.DoubleRow

NPF8 = ml_dtypes.float8_e4m3
NPBF = ml_dtypes.bfloat16

NBT = BL // 128  # 4 batch tiles per core
NK = D // 128    # 8 contraction tiles over D (== over H1)
NM = H1 // 128   # 8 output tiles over H1

SX = 16.0    # x scale before fp8
SW = 512.0   # weight scale before fp8
SH = 16.0    # hidden scale before fp8
S1 = SX * SW  # L1 PSUM units per true unit (8192)
S2 = SH * SW  # L2 PSUM units (8192)

NEXP = TOTAL_E  # device expert order: [shared0, shared1, spec0..spec5]


def _build_nc(mm_dt=None):
    from contextlib import ExitStack

    nc = bacc.Bacc("TRN2", target_bir_lowering=False, debug=False)

    # x order: 0 = x_shared, 1..3 = x0..x2
    xil = [
        nc.dram_tensor(f"xil{t}", [128, NK, 2, BL], F8, kind="ExternalInput")
        for t in range(4)
    ]
    w1a = nc.dram_tensor("w1a", [NEXP, NM, 128, NK, 2, 128], F8, kind="ExternalInput")
    w2a = nc.dram_tensor("w2a", [NEXP, 128, NK, 2, H2], F8, kind="ExternalInput")
    b1a = nc.dram_tensor("b1a", [128, NEXP, NM], F32, kind="ExternalInput")
    b2a = nc.dram_tensor("b2a", [1, NEXP, H2], BF16, kind="ExternalInput")
    wsga = nc.dram_tensor("wsga", [128, NK, 2, TOTAL_E], F8, kind="ExternalInput")
    wga = nc.dram_tensor("wga", [DOM, 128, NK, 2, GATE_K], F8, kind="ExternalInput")
    bsga = nc.dram_tensor("bsga", [1, TOTAL_E], BF16, kind="ExternalInput")
    bga = nc.dram_tensor("bga", [1, DOM, GATE_K], BF16, kind="ExternalInput")
    ys = [
        nc.dram_tensor(n, [BL, H2], BF16, kind="ExternalOutput")
        for n in ("y0", "y1", "y2", "ysh")
    ]

    with tile.TileContext(nc) as tc, ExitStack() as ctx:
        p_const = ctx.enter_context(tc.tile_pool(name="const", bufs=1))
        p_x = ctx.enter_context(tc.tile_pool(name="x", bufs=3))
        p_w1 = ctx.enter_context(tc.tile_pool(name="w1", bufs=2))
        p_w2 = ctx.enter_context(tc.tile_pool(name="w2", bufs=2))
        p_wg = ctx.enter_context(tc.tile_pool(name="wg", bufs=1))
        p_h = ctx.enter_context(tc.tile_pool(name="hT", bufs=3))
        p_hf = ctx.enter_context(tc.tile_pool(name="hf", bufs=4))
        p_oe = ctx.enter_context(tc.tile_pool(name="oe", bufs=2))
        p_osh = ctx.enter_context(tc.tile_pool(name="osh", bufs=1))
        p_acc = ctx.enter_context(tc.tile_pool(name="acc", bufs=1))
        p_bias = ctx.enter_context(tc.tile_pool(name="bias", bufs=1))
        p_gw = ctx.enter_context(tc.tile_pool(name="gw", bufs=1))
        p_sm = ctx.enter_context(tc.tile_pool(name="sm", bufs=3))
        ps_l1 = ctx.enter_context(tc.tile_pool(name="psl1", bufs=3, space="PSUM"))
        ps_l2 = ctx.enter_context(tc.tile_pool(name="psl2", bufs=3, space="PSUM"))
        ps_g = ctx.enter_context(tc.tile_pool(name="psg", bufs=2, space="PSUM"))

        onesb = p_const.tile([1, 128], BF16)
        nc.gpsimd.memset(onesb, 1.0)
        warm8 = p_const.tile([128, 2, 256], F8)
        nc.gpsimd.memset(warm8, 0.0)

        # PE warm-up: ~3us of dummy DoubleRow matmuls while the first DMAs
        # land, so the p-state ramp finishes before real work.
        for _ in range(28):
            pw = ps_g.tile([128, 512], F32, tag="pg", name="pw")
            nc.tensor.matmul(
                pw[:, :256], lhsT=warm8[:, :, :128], rhs=warm8,
                start=True, stop=True, perf_mode=DR,
            )

        # biases (small, early)
        b1t = p_bias.tile([128, NEXP, NM], F32, tag="b1")
        nc.sync.dma_start(out=b1t, in_=b1a[:])
        b2t = p_bias.tile([1, NEXP, H2], BF16, tag="b2")
        nc.sync.dma_start(out=b2t, in_=b2a[:])
        bsgt = p_bias.tile([1, TOTAL_E], BF16, tag="bsg")
        nc.sync.dma_start(out=bsgt, in_=bsga[:])
        bgt = p_bias.tile([1, DOM, GATE_K], BF16, tag="bg")
        nc.sync.dma_start(out=bgt, in_=bga[:])

        def load_x(t):
            xt = p_x.tile([128, NK, 2, BL], F8, tag="x")
            nc.sync.dma_start(out=xt, in_=xil[t][:])
            return xt

        def load_w1(e):
            w1t = p_w1.tile([128, NM, NK, 2, 128], F8, tag="w1")
            src = w1a[e].rearrange("m p k two j -> p m k two j")
            nc.sync.dma_start(out=w1t[:, 0 : NM // 2], in_=src[:, 0 : NM // 2])
            nc.sync.dma_start(out=w1t[:, NM // 2 :], in_=src[:, NM // 2 :])
            return w1t

        def load_w2(e):
            w2t = p_w2.tile([128, NK, 2, H2], F8, tag="w2")
            nc.sync.dma_start(out=w2t, in_=w2a[e])
            return w2t

        def gate(xt, wgt, bg_row, K, tag):
            """softmax(x @ Wg + bg) -> gw tile [128, NBT, K], b on partitions."""
            gw = p_gw.tile([128, NBT, K], F32, tag=tag)
            for bt in range(NBT):
                b0 = bt * 128
                pg = ps_g.tile([128, 512], F32, tag="pg", name="pg")
                nc.tensor.matmul(
                    pg[:, :K], lhsT=onesb, rhs=bg_row, start=True, stop=False,
                )
                for kt in range(NK):
                    nc.tensor.matmul(
                        pg[:, :K],
                        lhsT=xt[:, kt, :, b0 : b0 + 128],
                        rhs=wgt[:, kt, 0:1, :].broadcast_to([128, 2, K]),
                        start=False, stop=False, perf_mode=DR,
                    )
                for kp in range(NK // 2):
                    nc.tensor.matmul(
                        pg[:, :K],
                        lhsT=xt[:, 2 * kp : 2 * kp + 2, 0, b0 : b0 + 128],
                        rhs=wgt[:, 2 * kp : 2 * kp + 2, 1, :],
                        start=False, stop=(kp == NK // 2 - 1), perf_mode=DR,
                    )
                nm = p_sm.tile([128, 1], F32, tag="nm")
                nc.vector.reduce_max(out=nm, in_=pg[:, :K], axis=AX.X, negate=True)
                nms = p_sm.tile([128, 1], F32, tag="nms")
                nc.vector.tensor_scalar_mul(nms, nm, 1.0 / S1)
                esb = p_sm.tile([128, K], F32, tag=f"esb{K}")
                nc.scalar.activation(
                    out=esb, in_=pg[:, :K], func=AF.Exp, bias=nms, scale=1.0 / S1
                )
                ssb = p_sm.tile([128, 1], F32, tag="ssb")
                nc.vector.reduce_sum(out=ssb, in_=esb, axis=AX.X)
                rsb = p_sm.tile([128, 1], F32, tag="rsb")
                nc.vector.reciprocal(out=rsb, in_=ssb)
                nc.vector.tensor_scalar_mul(gw[:, bt, :], esb, rsb)
            return gw

        def expert_l1(e, xt, w1t):
            """L1: returns interleaved hT fp8 tile [128, NK, 2, BL]."""
            hT = p_h.tile([128, NK, 2, BL], F8, tag="hT")
            for mt in range(NM):
                hf = p_hf.tile([128, BL], F32, tag="hf")
                for cb in range(2):
                    c0 = cb * 256
                    pt = ps_l1.tile([128, 512], F32, tag="l1")
                    for kt in range(NK):
                        nc.tensor.matmul(
                            pt[:, :256],
                            lhsT=w1t[:, mt, kt, :, :],
                            rhs=xt[:, kt, 0:1, c0 : c0 + 256].broadcast_to(
                                [128, 2, 256]
                            ),
                            start=(kt == 0), stop=False, perf_mode=DR,
                        )
                    for kp in range(NK // 2):
                        nc.tensor.matmul(
                            pt[:, :256],
                            lhsT=w1t[:, mt, 2 * kp : 2 * kp + 2, 0, :],
                            rhs=xt[:, 2 * kp : 2 * kp + 2, 1, c0 : c0 + 256],
                            start=False, stop=(kp == NK // 2 - 1), perf_mode=DR,
                        )
                    nc.scalar.activation(
                        out=hf[:, c0 : c0 + 256], in_=pt[:, :256],
                        func=AF.Relu, bias=b1t[:, e, mt : mt + 1], scale=SH / S1,
                    )
                nc.gpsimd.tensor_copy(out=hT[:, mt, 0, :], in_=hf)
                nc.vector.tensor_tensor(
                    hT[:, mt, 1, :], hf, hT[:, mt, 0, :], ALU.subtract
                )
            return hT

        def expert_l2(e, hT, w2t, out_pool, tag):
            """L2: oe tile [128, NBT, H2] bf16 = relu(h @ W2 + b2)."""
            oe = out_pool.tile([128, NBT, H2], BF16, tag=tag)
            for bt in range(NBT):
                b0 = bt * 128
                for cb in range(2):
                    c0 = cb * 256
                    pt = ps_l2.tile([128, 512], F32, tag="l2")
                    nc.tensor.matmul(
                        pt[:, :256], lhsT=onesb, rhs=b2t[0:1, e, c0 : c0 + 256],
                        start=True, stop=False,
                    )
                    for kt in range(NK):
                        nc.tensor.matmul(
                            pt[:, :256],
                            lhsT=hT[:, kt, 0:1, b0 : b0 + 128].broadcast_to(
                                [128, 2, 128]
                            ),
                            rhs=w2t[:, kt, :, c0 : c0 + 256],
                            start=False, stop=False, perf_mode=DR,
                        )
                    for kp in range(NK // 2):
                        nc.tensor.matmul(
                            pt[:, :256],
                            lhsT=hT[:, 2 * kp : 2 * kp + 2, 1, b0 : b0 + 128],
                            rhs=w2t[:, 2 * kp : 2 * kp + 2, 0, c0 : c0 + 256],
                            start=False, stop=(kp == NK // 2 - 1), perf_mode=DR,
                        )
                    nc.scalar.activation(
                        out=oe[:, bt, c0 : c0 + 256], in_=pt[:, :256],
                        func=AF.Relu, scale=1.0 / S2,
                    )
            return oe

        accs = [None] * 4

        def accumulate(acc_idx, oe, gw, col, first):
            acc = accs[acc_idx]
            for bt in range(NBT):
                if first:
                    nc.vector.tensor_scalar_mul(
                        acc[:, bt, :], oe[:, bt, :], gw[:, bt, col : col + 1]
                    )
                else:
                    nc.vector.scalar_tensor_tensor(
                        out=acc[:, bt, :],
                        in0=oe[:, bt, :],
                        scalar=gw[:, bt, col : col + 1],
                        in1=acc[:, bt, :],
                        op0=ALU.mult,
                        op1=ALU.add,
                    )

        def store(acc_idx, y_dram):
            yr = y_dram[:].rearrange("(bt p) o -> bt p o", p=128)
            for bt in range(NBT):
                nc.sync.dma_start(out=yr[bt], in_=accs[acc_idx][:, bt, :])

        # ---- software-pipelined schedule ----
        xt_sh = load_x(0)
        wsgt = p_wg.tile([128, NK, 2, TOTAL_E], F8, tag="wsg")
        nc.sync.dma_start(out=wsgt, in_=wsga[:])
        wgts = []
        for d in range(DOM):
            wgt = p_wg.tile([128, NK, 2, GATE_K], F8, tag=f"wg{d}")
            nc.sync.dma_start(out=wgt, in_=wga[d])
            wgts.append(wgt)

        gws = gate(xt_sh, wsgt, bsgt, TOTAL_E, tag="gws")

        # shared expert 0
        w1t = load_w1(0)
        w2t0 = load_w2(0)
        hT0 = expert_l1(0, xt_sh, w1t)
        # shared expert 1
        w1t = load_w1(1)
        w2t1 = load_w2(1)
        xt0 = load_x(1)
        hT1 = expert_l1(1, xt_sh, w1t)
        osh0 = expert_l2(0, hT0, w2t0, p_osh, tag="osh0")
        gw0 = gate(xt0, wgts[0], bgt[0:1, 0, :], GATE_K, tag="gw0")

        for i in range(4):
            accs[i] = p_acc.tile(
                [128, NBT, H2], BF16, tag=f"acc{i}", name=f"acc{i}"
            )

        # spec e0 (device 2)
        w1t = load_w1(2)
        w2t2 = load_w2(2)
        hT2 = expert_l1(2, xt0, w1t)
        osh1 = expert_l2(1, hT1, w2t1, p_osh, tag="osh1")
        accumulate(3, osh0, gws, E_SPEC + 0, first=True)
        accumulate(3, osh1, gws, E_SPEC + 1, first=False)
        accumulate(0, osh0, gw0, NES + 0, first=True)
        accumulate(0, osh1, gw0, NES + 1, first=False)

        # spec e1 (device 3)
        w1t = load_w1(3)
        w2t3 = load_w2(3)
        xt1 = load_x(2)
        hT3 = expert_l1(3, xt0, w1t)
        oe = expert_l2(2, hT2, w2t2, p_oe, tag="oe")
        accumulate(0, oe, gw0, 0, first=False)
        accumulate(3, oe, gws, 0, first=False)
        gw1 = gate(xt1, wgts[1], bgt[0:1, 1, :], GATE_K, tag="gw1")

        # spec e2 (device 4)
        w1t = load_w1(4)
        w2t4 = load_w2(4)
        hT4 = expert_l1(4, xt1, w1t)
        oe = expert_l2(3, hT3, w2t3, p_oe, tag="oe")
        accumulate(0, oe, gw0, 1, first=False)
        accumulate(3, oe, gws, 1, first=False)
        store(0, ys[0])
        accumulate(1, osh0, gw1, NES + 0, first=True)
        accumulate(1, osh1, gw1, NES + 1, first=False)

        # spec e3 (device 5)
        w1t = load_w1(5)
        w2t5 = load_w2(5)
        xt2 = load_x(3)
        hT5 = expert_l1(5, xt1, w1t)
        oe = expert_l2(4, hT4, w2t4, p_oe, tag="oe")
        accumulate(1, oe, gw1, 0, first=False)
        accumulate(3, oe, gws, 2, first=False)
        gw2 = gate(xt2, wgts[2], bgt[0:1, 2, :], GATE_K, tag="gw2")

        # spec e4 (device 6)
        w1t = load_w1(6)
        w2t6 = load_w2(6)
        hT6 = expert_l1(6, xt2, w1t)
        oe = expert_l2(5, hT5, w2t5, p_oe, tag="oe")
        accumulate(1, oe, gw1, 1, first=False)
        accumulate(3, oe, gws, 3, first=False)
        store(1, ys[1])
        accumulate(2, osh0, gw2, NES + 0, first=True)
        accumulate(2, osh1, gw2, NES + 1, first=False)

        # spec e5 (device 7)
        w1t = load_w1(7)
        w2t7 = load_w2(7)
        hT7 = expert_l1(7, xt2, w1t)
        oe = expert_l2(6, hT6, w2t6, p_oe, tag="oe")
        accumulate(2, oe, gw2, 0, first=False)
        accumulate(3, oe, gws, 4, first=False)

        # tail
        oe = expert_l2(7, hT7, w2t7, p_oe, tag="oe")
        accumulate(2, oe, gw2, 1, first=False)
        accumulate(3, oe, gws, 5, first=False)
        store(2, ys[2])
        store(3, ys[3])

    nc.compile()
    return nc


_NC_CACHE = {}


def _get_nc(mm_dt=None):
    key = "fp8dr"
    if key not in _NC_CACHE:
        _NC_CACHE[key] = _build_nc()
    return _NC_CACHE[key]


def _hilo(a, s):
    af = np.asarray(a, np.float32) * np.float32(s)
    hi = af.astype(NPF8)
    lo = (af - hi.astype(np.float32)).astype(NPF8)
    return hi, lo


def _prep_inputs(inputs):
    """Quantize/layout all operands for the device (host-side prep)."""
    f = {k: np.asarray(v, np.float32) for k, v in inputs.items()}

    # x tensors: device order [x_shared, x0, x1, x2]
    x_full = [f["x_shared"], f["x0"], f["x1"], f["x2"]]
    x_per_core = []  # [t][core] -> [128, NK, 2, BL] fp8
    for x in x_full:
        hi, lo = _hilo(x, SX)              # [B, D]
        hi = hi.reshape(B, NK, 128)
        lo = lo.reshape(B, NK, 128)
        cores = []
        for c in range(N_CORES):
            sl = slice(c * BL, (c + 1) * BL)
            xa = np.empty((128, NK, 2, BL), NPF8)
            xa[:, :, 0, :] = hi[sl].transpose(2, 1, 0)
            xa[:, :, 1, :] = lo[sl].transpose(2, 1, 0)
            cores.append(xa)
        x_per_core.append(cores)

    # weights: device expert order [shared0, shared1, spec0..spec5]
    W1 = np.concatenate([f["W1h"], f["W1s"]], axis=0)  # [8, D, H1]
    W2 = np.concatenate([f["W2h"], f["W2s"]], axis=0)  # [8, H1, H2]
    b1 = np.concatenate([f["b1h"], f["b1s"]], axis=0)  # [8, H1]
    b2 = np.concatenate([f["b2h"], f["b2s"]], axis=0)  # [8, H2]

    h1i, l1i = _hilo(W1, SW)
    h1i = h1i.reshape(NEXP, NK, 128, NM, 128)
    l1i = l1i.reshape(NEXP, NK, 128, NM, 128)
    w1a = np.empty((NEXP, NM, 128, NK, 2, 128), NPF8)
    w1a[:, :, :, :, 0, :] = h1i.transpose(0, 3, 2, 1, 4)
    w1a[:, :, :, :, 1, :] = l1i.transpose(0, 3, 2, 1, 4)

    h2i, l2i = _hilo(W2, SW)
    h2i = h2i.reshape(NEXP, NK, 128, H2)
    l2i = l2i.reshape(NEXP, NK, 128, H2)
    w2a = np.empty((NEXP, 128, NK, 2, H2), NPF8)
    w2a[:, :, :, 0, :] = h2i.transpose(0, 2, 1, 3)
    w2a[:, :, :, 1, :] = l2i.transpose(0, 2, 1, 3)

    b1a = np.ascontiguousarray(
        (b1 * SH).reshape(NEXP, NM, 128).transpose(2, 0, 1), dtype=np.float32
    )
    b2a = (b2 * S2).reshape(1, NEXP, H2).astype(NPBF)

    hsg, lsg = _hilo(f["Wsg"], SW)  # [D, TOTAL_E]
    wsga = np.empty((128, NK, 2, TOTAL_E), NPF8)
    wsga[:, :, 0, :] = hsg.reshape(NK, 128, TOTAL_E).transpose(1, 0, 2)
    wsga[:, :, 1, :] = lsg.reshape(NK, 128, TOTAL_E).transpose(1, 0, 2)

    hg, lg = _hilo(f["Wg"], SW)  # [DOM, D, GATE_K]
    wga = np.empty((DOM, 128, NK, 2, GATE_K), NPF8)
    wga[:, :, :, 0, :] = hg.reshape(DOM, NK, 128, GATE_K).transpose(0, 2, 1, 3)
    wga[:, :, :, 1, :] = lg.reshape(DOM, NK, 128, GATE_K).transpose(0, 2, 1, 3)

    bsga = (f["bsg"] * S1).reshape(1, TOTAL_E).astype(NPBF)
    bga = (f["bg"] * S1).reshape(1, DOM, GATE_K).astype(NPBF)

    shared = {
        "w1a": w1a, "w2a": w2a, "b1a": b1a, "b2a": b2a,
        "wsga": wsga, "wga": wga, "bsga": bsga, "bga": bga,
    }
    in_maps = []
    for c in range(N_CORES):
        m = dict(shared)
        for t in range(4):
            m[f"xil{t}"] = x_per_core[t][c]
        in_maps.append(m)
    return in_maps


def kernel(**inputs):
    return run_kernel(inputs)


def run_kernel(inputs, mm_dt=None, trace=False):
    nc = _get_nc()
    in_maps = _prep_inputs(inputs)
    res = run_bass_kernel_spmd(nc, in_maps, list(range(N_CORES)), trace=trace)
    outs = []
    for name in ("y0", "y1", "y2", "ysh"):
        outs.append(
            np.concatenate(
                [
                    np.asarray(res.results[c][name]).astype(np.float32)
                    for c in range(N_CORES)
                ],
                axis=0,
            )
        )
    out = tuple(outs)
    if trace:
        return out, res
    return out
